# revision 1
# baseline (speedup 1.0000x reference)
"""MeanAggregator (GNN segment-mean) Bass kernel for 8 Trainium2 NeuronCores.

Reference computation:
    gathered = features[edge_dst]                       # [E, D]
    sums     = segment_sum(gathered, edge_seg, B)       # [B, D]
    counts   = segment_sum(ones(E), edge_seg, B)        # [B]
    out      = sums / counts[:, None]                   # [B, D]

Strategy: shard output nodes (segments) contiguously across the 8 cores;
edge_seg is sorted, so each core owns its output rows outright -- no
collectives.

Fast path (v2, uniform degree): per 128-node tile the neighbor rows are
gathered with the GpSimd `dma_gather` ucode (CounterMachine descriptor
generation -- ~2x the row rate of generic indirect DMA, whose Q7
software descriptor loop serializes the whole kernel).  dma_gather takes
int16 indices, so each tile's (row, node) entries are sorted by row id
and bucketed into 32768-row windows (index relative to window base);
unused trailing slots are -1, which the ucode skips for free.  The
gathered rows land in sorted order, not per-node order, so a per-block
[128x128] 0/1 selection matrix (one VectorE is_equal against an iota
constant) and a PE matmul accumulate each block into the [node, D] PSUM
tile -- the permutation is absorbed by the matmul on an otherwise idle
engine.  The self-loop column is a contiguous slab, loaded at line rate
by HWDGE instead of being gathered.  Fallback (v1, arbitrary counts):
per-column indirect DMAs + VectorE tree reduction with per-edge weights.
"""

import sys

for _p in ("/opt/trn_rl_repo", "/root/.axon_site/_ro/trn_rl_repo"):
    if _p not in sys.path:
        sys.path.append(_p)

import numpy as np

from concourse import bacc, bass, mybir
import concourse.tile as tile
from concourse.bass_utils import run_bass_kernel_spmd

TRACE = False            # set by test.py to profile the HW run
TRACE_KWARGS = {"trace": True}
LAST_RESULT = None

P = 128          # SBUF partitions = nodes per tile
D = 128          # feature dim
N_CORES = 8
N_TOTAL = 100000  # feature table rows


def build_program(n_tiles: int, K: int, weighted: bool,
                  g_bufs: int = 3, repeats: int = 1) -> bass.Bass:
    """Bass program run identically on every core.

    Inputs per core:
      features [N_TOTAL, D] f32  (replicated)
      idx      [n_tiles*P, K] i32  (this core's neighbor ids, padded)
      wts      [n_tiles*P, K] f32  (only if weighted: per-edge weight, e.g.
                                    1/count with 0 for padding)
    Output per core:
      out      [n_tiles*P, D] f32
    """
    nodes = n_tiles * P
    nc = bacc.Bacc("TRN2", target_bir_lowering=False)
    feat = nc.declare_dram_parameter("features", [N_TOTAL, D],
                                     mybir.dt.float32, isOutput=False)
    idx = nc.declare_dram_parameter("idx", [nodes, K],
                                    mybir.dt.int32, isOutput=False)
    if weighted:
        wts = nc.declare_dram_parameter("wts", [nodes, K],
                                        mybir.dt.float32, isOutput=False)
    out = nc.declare_dram_parameter("out", [nodes, D],
                                    mybir.dt.float32, isOutput=True)

    with tile.TileContext(nc) as tc:
        with tc.tile_pool(name="gath", bufs=g_bufs) as gp, \
             tc.tile_pool(name="io", bufs=4) as iop, \
             tc.tile_pool(name="res", bufs=4) as rp:
            for t in range(n_tiles * repeats):
                t = t % n_tiles
                sl = slice(t * P, (t + 1) * P)
                idx_t = iop.tile([P, K], mybir.dt.int32, tag="idx")
                nc.sync.dma_start(out=idx_t[:], in_=idx[sl, :])
                G = gp.tile([P, K * D], mybir.dt.float32, tag="g")
                # NOTE: one indirect DMA per neighbor column. A single DMA
                # with a [P, K] offset AP is NOT equivalent on real HW (the
                # DGE scrambles multi-index-per-partition gathers).
                for j in range(K):
                    nc.gpsimd.indirect_dma_start(
                        out=G[:, j * D:(j + 1) * D],
                        out_offset=None,
                        in_=feat[:],
                        in_offset=bass.IndirectOffsetOnAxis(
                            ap=idx_t[:, j:j + 1], axis=0),
                    )
                if weighted:
                    w_t = iop.tile([P, K], mybir.dt.float32, tag="w")
                    nc.sync.dma_start(out=w_t[:], in_=wts[sl, :])
                    for j in range(K):
                        nc.vector.tensor_scalar_mul(
                            out=G[:, j * D:(j + 1) * D],
                            in0=G[:, j * D:(j + 1) * D],
                            scalar1=w_t[:, j:j + 1],
                        )
                # pairwise in-place tree reduction of the K blocks of G
                cur = K
                while cur > 1:
                    h = cur // 2
                    nc.vector.tensor_tensor(
                        out=G[:, :h * D],
                        in0=G[:, :h * D],
                        in1=G[:, h * D:2 * h * D],
                        op=mybir.AluOpType.add,
                    )
                    if cur % 2:
                        nc.vector.tensor_tensor(
                            out=G[:, (h - 1) * D:h * D],
                            in0=G[:, (h - 1) * D:h * D],
                            in1=G[:, (cur - 1) * D:cur * D],
                            op=mybir.AluOpType.add,
                        )
                    cur = h
                o_t = rp.tile([P, D], mybir.dt.float32, tag="o")
                if weighted:
                    # weights already include the 1/count factor
                    nc.vector.tensor_copy(out=o_t[:], in_=G[:, :D])
                else:
                    nc.vector.tensor_scalar_mul(out=o_t[:], in0=G[:, :D],
                                                scalar1=1.0 / K)
                nc.sync.dma_start(out=out[sl, :], in_=o_t[:])
    nc.compile()
    return nc


WINDOW = 32768
N_WINDOWS = 4            # ceil(100000 / 32768)


V2_CHUNK = 7             # max blocks (x128 rows) per dma_gather call
V2_QUEUES = 4            # SWDGE queues to round-robin gather calls over


def _v2_layout(Kg):
    """Static per-tile block/call layout for entries = P*Kg sorted rows.

    Returns (blocks_per_window, call list [(w, b0, nb)], TB)."""
    ent = P * Kg
    blocks = []
    for w in range(N_WINDOWS):
        wsize = min(WINDOW, N_TOTAL - w * WINDOW)
        p = wsize / N_TOTAL
        mean = ent * p
        sigma = (ent * p * (1 - p)) ** 0.5
        need = mean + 8 * sigma + 8
        blocks.append(max(1, -(-int(need) // P)))
    calls = []
    off = 0
    for w, bw in enumerate(blocks):
        b0 = off
        while b0 < off + bw:
            nb = min(V2_CHUNK, off + bw - b0)
            calls.append((w, b0, nb))
            b0 += nb
        off += bw
    return blocks, calls, sum(blocks)


def build_program_v2(n_tiles, Kg, scale, g_bufs=3, repeats=1,
                     min_wcounts=None):
    """Fast path: dma_gather (value-sorted, windowed int16 indices) +
    selection-matmul un-permutation on PE + contiguous self-loop slab.

    min_wcounts: per-window minimum valid-entry count over all tiles/cores
    (from host prep).  Blocks below that watermark are written by every
    tile's gather; only blocks above it need the NaN-safety memset."""
    blocks, calls, TB = _v2_layout(Kg)
    NCALLS = len(calls)
    nodes = n_tiles * P
    woff = np.cumsum([0] + blocks[:-1])     # block offset of each window

    nc = bacc.Bacc("TRN2", target_bir_lowering=False,
                   num_swdge_queues=V2_QUEUES)
    feat = nc.declare_dram_parameter("features", [N_TOTAL, D],
                                     mybir.dt.float32, isOutput=False)
    sl = nc.declare_dram_parameter("selfloop", [nodes, D],
                                   mybir.dt.float32, isOutput=False)
    idx = nc.declare_dram_parameter("idx16", [n_tiles, 128, TB * 8],
                                    mybir.dt.int16, isOutput=False)
    nid = nc.declare_dram_parameter("nodeids", [n_tiles, 128, TB],
                                    mybir.dt.float32, isOutput=False)
    cnt = nc.declare_dram_parameter("counts", [1, n_tiles * NCALLS],
                                    mybir.dt.int32, isOutput=False)
    out = nc.declare_dram_parameter("out", [nodes, D],
                                    mybir.dt.float32, isOutput=True)

    with tile.TileContext(nc) as tc:
        with tc.tile_pool(name="const", bufs=1) as cp, \
             tc.tile_pool(name="io", bufs=4) as iop, \
             tc.tile_pool(name="gath", bufs=g_bufs) as gp, \
             tc.tile_pool(name="sel", bufs=4) as sp_, \
             tc.tile_pool(name="ps", bufs=4, space="PSUM") as pp, \
             tc.tile_pool(name="res", bufs=4) as rp:
            iota_i = cp.tile([P, P], mybir.dt.int32)
            nc.gpsimd.iota(iota_i[:], pattern=[[1, P]], channel_multiplier=0)
            iota_f = cp.tile([P, P], mybir.dt.float32)
            nc.vector.tensor_copy(out=iota_f[:], in_=iota_i[:])
            # all per-call valid counts, loaded once (a persistent tile: the
            # per-call register loads below are not tracked as tile reads,
            # so a recycled per-tile buffer would race with them)
            cnt_all = cp.tile([1, n_tiles * NCALLS], mybir.dt.int32)
            nc.sync.dma_start(out=cnt_all[:], in_=cnt[:, :])
            with nc.gpsimd.register("rcnt") as rcnt:
                for t in range(n_tiles * repeats):
                    t = t % n_tiles
                    idx_t = iop.tile([128, TB * 8], mybir.dt.int16, tag="idx")
                    nc.sync.dma_start(out=idx_t[:], in_=idx[t, :, :])
                    nid_t = iop.tile([128, TB], mybir.dt.float32, tag="nid")
                    nc.sync.dma_start(out=nid_t[:], in_=nid[t, :, :])
                    sl_t = iop.tile([P, D], mybir.dt.float32, tag="sl")
                    nc.sync.dma_start(out=sl_t[:],
                                      in_=sl[t * P:(t + 1) * P, :])
                    G = gp.tile([P, TB * D], mybir.dt.float32, tag="g")
                    # clear blocks the gather may leave unwritten
                    # (tail-skipped entries): NaN x 0-sel would poison psum
                    for w in range(N_WINDOWS):
                        gw = 0 if min_wcounts is None else \
                            int(min_wcounts[w]) // P
                        lo_b, hi_b = int(woff[w]) + gw, \
                            int(woff[w]) + blocks[w]
                        if hi_b > lo_b:
                            nc.vector.memset(
                                G[:, lo_b * D:hi_b * D], 0)
                    for c, (w, b0, nb) in enumerate(calls):
                        wbase = w * WINDOW
                        wsize = min(WINDOW, N_TOTAL - wbase)
                        cslot = t * NCALLS + c
                        nc.gpsimd.load(rcnt, cnt_all[0:1, cslot:cslot + 1])
                        nc.gpsimd.dma_gather(
                            out_ap=G[:, b0 * D:(b0 + nb) * D].rearrange(
                                "p (b d) -> p b d", d=D),
                            in_ap=feat[wbase:wbase + wsize],
                            idxs_ap=idx_t[:, b0 * 8:(b0 + nb) * 8],
                            num_idxs=nb * P,
                            num_idxs_reg=rcnt,
                            elem_size=D,
                            queue_num=(t * len(calls) + c) % V2_QUEUES,
                        )
                    ps = pp.tile([P, D], mybir.dt.float32, tag="ps")
                    for b in range(TB):
                        sel = sp_.tile([P, P], mybir.dt.float32, tag="sel")
                        nc.vector.tensor_tensor(
                            out=sel[:],
                            in0=nid_t[:, b:b + 1].to_broadcast([P, P]),
                            in1=iota_f[:],
                            op=mybir.AluOpType.is_equal,
                        )
                        nc.tensor.matmul(
                            out=ps[:],
                            lhsT=sel[:],
                            rhs=G[:, b * D:(b + 1) * D],
                            start=(b == 0),
                            stop=(b == TB - 1),
                        )
                    o_t = rp.tile([P, D], mybir.dt.float32, tag="o")
                    nc.vector.tensor_tensor(out=o_t[:], in0=ps[:],
                                            in1=sl_t[:],
                                            op=mybir.AluOpType.add)
                    nc.vector.tensor_scalar_mul(out=o_t[:], in0=o_t[:],
                                                scalar1=scale)
                    nc.sync.dma_start(out=out[t * P:(t + 1) * P, :],
                                      in_=o_t[:])
    nc.compile()
    return nc


def _prep_v2(idx_cols, n_tiles):
    """Host prep for one core.  idx_cols [n_tiles*P, Kg] int32 row ids.

    Returns idx16 [n_tiles,128,TB*8] i16 (wrapped + 8x replicated),
    nodeids [n_tiles,128,TB] f32, counts [n_tiles,NCALLS] i32."""
    Kg = idx_cols.shape[1]
    blocks, calls, TB = _v2_layout(Kg)
    woff = np.cumsum([0] + blocks[:-1])
    NCALLS = len(calls)
    idx16 = np.zeros((n_tiles, 128, TB * 8), np.int16)
    nodeids = np.zeros((n_tiles, 128, TB), np.float32)
    counts = np.zeros((n_tiles, NCALLS), np.int32)
    wmin = [10 ** 9] * N_WINDOWS
    # counts is reshaped to [1, n_tiles*NCALLS] for the program input
    wbounds = [w * WINDOW for w in range(N_WINDOWS + 1)]
    wbounds[-1] = N_TOTAL + 1
    for t in range(n_tiles):
        rf = idx_cols[t * P:(t + 1) * P]
        r = rf.ravel().astype(np.int64)
        p = np.repeat(np.arange(P, dtype=np.int64), Kg)
        keep = r >= 0                  # pad rows are marked -1: not gathered
        r, p = r[keep], p[keep]
        o = np.argsort(r, kind="stable")
        rs, ps = r[o], p[o]
        slots_i = np.full(TB * P, -1, np.int64)
        slots_n = np.full(TB * P, -1.0, np.float32)
        bounds = np.searchsorted(rs, wbounds)
        n_w = [0] * N_WINDOWS
        for w in range(N_WINDOWS):
            s0, s1 = bounds[w], bounds[w + 1]
            n_w[w] = s1 - s0
            if n_w[w] > blocks[w] * P:
                raise OverflowError("v2 window block overflow")
            off = woff[w] * P
            slots_i[off:off + n_w[w]] = rs[s0:s1] - w * WINDOW
            slots_n[off:off + n_w[w]] = ps[s0:s1]
            wmin[w] = min(wmin[w], n_w[w])
        for c, (w, b0, nb) in enumerate(calls):
            c0 = (b0 - woff[w]) * P
            cc = min(max(n_w[w] - c0, 0), nb * P)
            if cc == 0:
                slots_i[b0 * P] = 0      # dummy valid entry, nid stays -1
                cc = 1
            counts[t, c] = cc
        nodeids[t] = slots_i_to_mat = slots_n.reshape(TB, P).T
        for (w, b0, nb) in calls:
            arr = slots_i[b0 * P:(b0 + nb) * P].reshape(nb * 8, 16)
            idx16[t, :16, b0 * 8:(b0 + nb) * 8] = arr.T.astype(np.int16)
        idx16[t] = np.tile(idx16[t, :16], (8, 1))
    return idx16, nodeids, counts.reshape(1, -1), wmin


def _prep_general(edge_seg, edge_dst, B):
    """Arbitrary sorted-or-not edge_seg: build padded [B, K] index and
    weight matrices (weight = 1/count, 0 on padding)."""
    E = edge_dst.shape[0]
    order = np.argsort(edge_seg, kind="stable")
    sseg = edge_seg[order].astype(np.int64)
    sdst = edge_dst[order].astype(np.int32)
    counts = np.bincount(sseg, minlength=B).astype(np.int64)
    K = max(int(counts.max()), 1) if E else 1
    starts = np.zeros(B, np.int64)
    np.cumsum(counts[:-1], out=starts[1:])
    pos = np.arange(E, dtype=np.int64) - np.repeat(starts, counts)
    idx_mat = np.zeros((B, K), np.int32)
    wts_mat = np.zeros((B, K), np.float32)
    idx_mat[sseg, pos] = sdst
    inv = np.zeros(B, np.float32)
    nz = counts > 0
    inv[nz] = 1.0 / counts[nz]
    wts_mat[sseg, pos] = inv[sseg]
    return idx_mat, wts_mat, K


def kernel(features, edge_seg, edge_dst, num_nodes=None, **_unused):
    features = np.ascontiguousarray(np.asarray(features, dtype=np.float32))
    edge_seg = np.asarray(edge_seg)
    edge_dst = np.asarray(edge_dst)
    E = int(edge_dst.shape[0])
    if num_nodes is not None:
        B = int(np.asarray(num_nodes))
    else:
        B = int(edge_seg.max()) + 1

    # Fast path: canonical uniform-degree layout (reference's setup_inputs):
    # edge_seg == repeat(arange(B), K) -> just reshape edge_dst.
    K = E // B if B and E % B == 0 else 0
    uniform = K > 0 and np.array_equal(
        edge_seg, np.repeat(np.arange(B, dtype=edge_seg.dtype), K))
    if uniform:
        idx_mat = np.ascontiguousarray(edge_dst.reshape(B, K).astype(np.int32))
        wts_mat = None
    else:
        idx_mat, wts_mat, K = _prep_general(edge_seg, edge_dst, B)

    # Shard rows contiguously across cores; pad each shard to a tile multiple.
    npc = -(-B // N_CORES)           # nodes per core (ceil)
    n_tiles = -(-npc // P)
    nodes_pad = n_tiles * P
    weighted = wts_mat is not None

    in_maps = None
    if not weighted:
        try:
            in_maps = prep_v2_in_maps(features, idx_mat, B, npc, n_tiles)
            nc = build_program_v2(n_tiles, in_maps[0]["_Kg"],
                                  scale=1.0 / K,
                                  min_wcounts=in_maps[0]["_wmin"])
            for m in in_maps:
                del m["_Kg"], m["_wmin"]
        except OverflowError:
            in_maps = None

    if in_maps is None:
        nc = build_program(n_tiles, K, weighted)
        in_maps = []
        for c in range(N_CORES):
            lo = c * npc
            hi = min(B, (c + 1) * npc)
            idx_c = np.zeros((nodes_pad, K), np.int32)
            if hi > lo:
                idx_c[:hi - lo] = idx_mat[lo:hi]
            m = {"features": features, "idx": idx_c}
            if weighted:
                w_c = np.zeros((nodes_pad, K), np.float32)
                if hi > lo:
                    w_c[:hi - lo] = wts_mat[lo:hi]
                m["wts"] = w_c
            in_maps.append(m)

    kw = dict(TRACE_KWARGS) if TRACE else {}
    res = run_bass_kernel_spmd(nc, in_maps, list(range(N_CORES)), **kw)
    global LAST_RESULT
    LAST_RESULT = res
    parts = []
    for c in range(N_CORES):
        lo = c * npc
        hi = min(B, (c + 1) * npc)
        if hi > lo:
            parts.append(res.results[c]["out"][:hi - lo])
    return np.concatenate(parts, axis=0)


def prep_v2_in_maps(features, idx_mat, B, npc, n_tiles):
    """Host prep for the v2 fast path: per-core inputs."""
    K = idx_mat.shape[1]
    selfloop = np.array_equal(idx_mat[:, 0],
                              np.arange(B, dtype=idx_mat.dtype))
    Kg = K - 1 if selfloop else K
    nodes_pad = n_tiles * P
    in_maps = []
    for c in range(N_CORES):
        lo = c * npc
        hi = min(B, (c + 1) * npc)
        cols = np.full((nodes_pad, Kg), -1, np.int32)
        sl_c = np.zeros((nodes_pad, D), np.float32)
        if hi > lo:
            cols[:hi - lo] = idx_mat[lo:hi, 1:] if selfloop \
                else idx_mat[lo:hi]
            if selfloop:
                sl_c[:hi - lo] = features[lo:hi]
        idx16, nodeids, counts, wmin = _prep_v2(cols, n_tiles)
        in_maps.append({"features": features, "selfloop": sl_c,
                        "idx16": idx16, "nodeids": nodeids,
                        "counts": counts, "_Kg": Kg, "_wmin": wmin})
    all_wmin = [min(m["_wmin"][w] for m in in_maps)
                for w in range(N_WINDOWS)]
    for m in in_maps:
        m["_wmin"] = all_wmin
    return in_maps



# revision 9
# speedup vs baseline: 2.5372x; 2.5372x over previous
"""MeanAggregator (GNN segment-mean) Bass kernel for 8 Trainium2 NeuronCores.

Reference computation:
    gathered = features[edge_dst]                       # [E, D]
    sums     = segment_sum(gathered, edge_seg, B)       # [B, D]
    counts   = segment_sum(ones(E), edge_seg, B)        # [B]
    out      = sums / counts[:, None]                   # [B, D]

Strategy: shard output nodes (segments) contiguously across the 8 cores;
edge_seg is sorted, so each core owns its output rows outright -- no
collectives.

v3 fast path (uniform degree K with a self-loop column):
  * features are converted to bf16 host-side: halves the gather bytes and
    doubles the PE matmul rate; the 2e-2 tolerance leaves bf16's ~4e-3
    mean-of-17 error a wide margin.
  * per 128-node tile the neighbor rows are fetched with the GpSimd
    `dma_gather` ucode.  Indices are int16 relative to one of four 32768-row
    windows; per (tile, window) the entries are sorted by row id and padded
    to a 128-slot block boundary with dummy index 0 / node-id -1 (every slot
    is always written -- no NaN hazard, no memsets, no count registers).
    Block counts are exact for the data at hand (the layout is built at
    kernel() time), not statistical bounds.
  * gather calls are batched across a GROUP of tiles (one call per
    (group, window)) to amortize the ~1us fixed SWDGE cost per call; the
    SWDGE descriptor ring is enlarged via dynamic_dma_scratch_size.
  * the gathered rows land row-sorted, not per-node; a per-block [128,128]
    bf16 0/1 selection matrix routes them: out[node] += sel.T @ G_block on
    the PE.  sel is built per block with ONE VectorE tensor_scalar
    (iota is_equal nid-scalar, x 1/K folded in) -- unit-stride 16-bit
    operands so the DVE 2x perf mode applies.
  * the self-loop column is a contiguous slab (HWDGE line rate) folded in
    with a scaled-identity matmul; PSUM is drained by the otherwise idle
    Activation engine into a per-group buffer, written back with one DMA.

Fallback (v1, arbitrary counts): per-column indirect DMAs + VectorE tree
reduction with per-edge weights.
"""

import sys

for _p in ("/opt/trn_rl_repo", "/root/.axon_site/_ro/trn_rl_repo"):
    if _p not in sys.path:
        sys.path.append(_p)

import numpy as np

from concourse import bacc, bass, mybir
import concourse.tile as tile
from concourse.bass_utils import run_bass_kernel_spmd

TRACE = False            # set by test.py to profile the HW run
TRACE_KWARGS = {"trace": True}
LAST_RESULT = None

P = 128          # SBUF partitions = nodes per tile
D = 128          # feature dim
N_CORES = 8
N_TOTAL = 100000  # feature table rows

WINDOW = 32768
N_WINDOWS = 4            # ceil(100000 / 32768)

GROUP_TILES = 6          # tiles per gather group
MAX_CALL_BLOCKS = 42     # max 128-row blocks per dma_gather call
V3_QUEUES = 4            # SWDGE queues to round-robin gather calls over
SINGLE_PACKET = False    # True packs a call into one packet but caps it at
                         # 64 descs = 1024 idxs; False allows larger calls


def _scratch_size():
    """SWDGE ring must hold one call's descriptors (16 B/desc mirror of
    ucode); the ring is a power-of-two circular buffer."""
    # single_packet ucode packs 16 idxs/descriptor -> MCB*8 descs + slack
    need = 16 * (MAX_CALL_BLOCKS * 8 + 64)
    sz = 16384
    while sz < need:
        sz *= 2
    return sz


def _bf16():
    return mybir.dt.np(mybir.dt.bfloat16)


# --------------------------------------------------------------------------
# v3 host-side planning


class _V3Plan:
    """Static per-core program layout, derived from the actual indices."""

    __slots__ = ("groups", "meta", "tbg_max", "meta_max", "n_tiles", "scale")

    def __init__(self):
        self.groups = []


def build_program_v3(plan, repeats: int = 1) -> bass.Bass:
    """Bass program run identically on every core.

    Inputs per core:
      features [N_TOTAL, D] bf16  (replicated table; also the self-loop rows)
      meta     [128, meta_total] i16 (idx words + nid columns per group)
      base     not needed -- self-loop rows are core-local, so each core's
               meta/out simply address its own slice; the slab reads use
               feat rows [r0_core + ...], baked per core?  No: feat is the
               SAME replicated table; slab rows differ per core.  We bake
               the core's row base into a per-core `sl` input instead.
      sl       [n_tiles*P, D] bf16 (this core's self-loop rows, contiguous)
    Output per core:
      out      [n_tiles*P, D] f32
    """
    n_tiles = plan.n_tiles
    nodes = n_tiles * P
    bf16 = mybir.dt.bfloat16
    nc = bacc.Bacc("TRN2", target_bir_lowering=False,
                   num_swdge_queues=V3_QUEUES,
                   dynamic_dma_scratch_size=_scratch_size())
    feat = nc.declare_dram_parameter("features", [N_TOTAL, D],
                                     bf16, isOutput=False)
    sl = nc.declare_dram_parameter("selfloop", [nodes, D],
                                   bf16, isOutput=False)
    meta = nc.declare_dram_parameter("meta", [128, plan.meta.shape[1]],
                                     mybir.dt.int16, isOutput=False)
    out = nc.declare_dram_parameter("out", [nodes, D],
                                    mybir.dt.float32, isOutput=True)
    gt_max = max(g["gt"] for g in plan.groups)

    with tile.TileContext(nc) as tc:
        with tc.tile_pool(name="const", bufs=1) as cp, \
             tc.tile_pool(name="meta", bufs=3) as mp, \
             tc.tile_pool(name="slab", bufs=3) as slp, \
             tc.tile_pool(name="gath", bufs=2) as gp, \
             tc.tile_pool(name="sel", bufs=8) as sp_, \
             tc.tile_pool(name="ps", bufs=4, space="PSUM") as pp, \
             tc.tile_pool(name="res", bufs=3) as rp:
            # iota_row[p, q] = q ; piota[p, q] = p      (int16)
            iota_row = cp.tile([P, P], mybir.dt.int16)
            nc.gpsimd.iota(iota_row[:], pattern=[[1, P]], channel_multiplier=0)
            piota_i = cp.tile([P, P], mybir.dt.int32)
            nc.gpsimd.iota(piota_i[:], pattern=[[0, P]],
                           channel_multiplier=1)
            piota = cp.tile([P, P], mybir.dt.float32)
            nc.vector.tensor_copy(out=piota[:], in_=piota_i[:])
            # I_scaled[p, q] = (q == p) * scale         (bf16)
            i_scaled = cp.tile([P, P], bf16)
            nc.vector.tensor_scalar(
                out=i_scaled[:], in0=iota_row[:],
                scalar1=piota[:, 0:1], scalar2=float(plan.scale),
                op0=mybir.AluOpType.is_equal, op1=mybir.AluOpType.mult)
            for rep in range(repeats):
                for g in plan.groups:
                    gt, tbg = g["gt"], g["tbg"]
                    meta_t = mp.tile([128, plan.meta_max], mybir.dt.int16,
                                     tag="meta")
                    nc.sync.dma_start(
                        out=meta_t[:, :g["glen"]],
                        in_=meta[:, g["goff"]:g["goff"] + g["glen"]])
                    slab_t = slp.tile([P, gt_max * D], bf16, tag="slab")
                    nc.scalar.dma_start(
                        out=slab_t[:, :gt * D].rearrange(
                            "p (b d) -> p b d", d=D),
                        in_=sl[g["r0"]:g["r0"] + gt * P, :].rearrange(
                            "(b p) d -> p b d", p=P))
                    G = gp.tile([P, plan.tbg_max * D], bf16, tag="g")
                    for c, (w, b0, nb, iw0) in enumerate(g["calls"]):
                        wbase = w * WINDOW
                        wsize = min(WINDOW, N_TOTAL - wbase)
                        nc.gpsimd.dma_gather(
                            out_ap=G[:, b0 * D:(b0 + nb) * D].rearrange(
                                "p (b d) -> p b d", d=D),
                            in_ap=feat[wbase:wbase + wsize],
                            idxs_ap=meta_t[:, iw0:iw0 + nb * 8],
                            num_idxs=nb * P,
                            num_idxs_reg=nb * P,
                            elem_size=D,
                            single_packet=SINGLE_PACKET,
                            queue_num=c % V3_QUEUES,
                        )
                    obuf = rp.tile([P, gt_max * D], mybir.dt.float32,
                                   tag="o")
                    for (ti, mm) in g["tiles"]:
                        ps = pp.tile([P, D], mybir.dt.float32, tag="ps")
                        nc.tensor.matmul(
                            out=ps[:], lhsT=i_scaled[:],
                            rhs=slab_t[:, ti * D:(ti + 1) * D],
                            start=True, stop=(len(mm) == 0))
                        for j, (gblk, nidcol) in enumerate(mm):
                            sel = sp_.tile([P, P], bf16, tag="sel")
                            nc.vector.tensor_scalar(
                                out=sel[:], in0=iota_row[:],
                                scalar1=meta_t[:, nidcol:nidcol + 2]
                                .bitcast(mybir.dt.float32),
                                scalar2=float(plan.scale),
                                op0=mybir.AluOpType.is_equal,
                                op1=mybir.AluOpType.mult)
                            nc.tensor.matmul(
                                out=ps[:], lhsT=sel[:],
                                rhs=G[:, gblk * D:(gblk + 1) * D],
                                start=False, stop=(j == len(mm) - 1))
                        nc.scalar.copy(out=obuf[:, ti * D:(ti + 1) * D],
                                       in_=ps[:])
                    nc.sync.dma_start(
                        out=out[g["r0"]:g["r0"] + gt * P, :].rearrange(
                            "(b p) d -> p b d", p=P),
                        in_=obuf[:, :gt * D].rearrange(
                            "p (b d) -> p b d", d=D))
    nc.compile()
    return nc


# --------------------------------------------------------------------------
# v1 fallback: arbitrary sorted-or-not edge_seg


def build_program(n_tiles: int, K: int, weighted: bool,
                  g_bufs: int = 3, repeats: int = 1) -> bass.Bass:
    """Fallback program (per-column indirect DMA + tree reduction)."""
    nodes = n_tiles * P
    nc = bacc.Bacc("TRN2", target_bir_lowering=False)
    feat = nc.declare_dram_parameter("features", [N_TOTAL, D],
                                     mybir.dt.float32, isOutput=False)
    idx = nc.declare_dram_parameter("idx", [nodes, K],
                                    mybir.dt.int32, isOutput=False)
    if weighted:
        wts = nc.declare_dram_parameter("wts", [nodes, K],
                                        mybir.dt.float32, isOutput=False)
    out = nc.declare_dram_parameter("out", [nodes, D],
                                    mybir.dt.float32, isOutput=True)

    with tile.TileContext(nc) as tc:
        with tc.tile_pool(name="gath", bufs=g_bufs) as gp, \
             tc.tile_pool(name="io", bufs=4) as iop, \
             tc.tile_pool(name="res", bufs=4) as rp:
            for t in range(n_tiles * repeats):
                t = t % n_tiles
                sl = slice(t * P, (t + 1) * P)
                idx_t = iop.tile([P, K], mybir.dt.int32, tag="idx")
                nc.sync.dma_start(out=idx_t[:], in_=idx[sl, :])
                G = gp.tile([P, K * D], mybir.dt.float32, tag="g")
                for j in range(K):
                    nc.gpsimd.indirect_dma_start(
                        out=G[:, j * D:(j + 1) * D],
                        out_offset=None,
                        in_=feat[:],
                        in_offset=bass.IndirectOffsetOnAxis(
                            ap=idx_t[:, j:j + 1], axis=0),
                    )
                if weighted:
                    w_t = iop.tile([P, K], mybir.dt.float32, tag="w")
                    nc.sync.dma_start(out=w_t[:], in_=wts[sl, :])
                    for j in range(K):
                        nc.vector.tensor_scalar_mul(
                            out=G[:, j * D:(j + 1) * D],
                            in0=G[:, j * D:(j + 1) * D],
                            scalar1=w_t[:, j:j + 1],
                        )
                cur = K
                while cur > 1:
                    h = cur // 2
                    nc.vector.tensor_tensor(
                        out=G[:, :h * D],
                        in0=G[:, :h * D],
                        in1=G[:, h * D:2 * h * D],
                        op=mybir.AluOpType.add,
                    )
                    if cur % 2:
                        nc.vector.tensor_tensor(
                            out=G[:, (h - 1) * D:h * D],
                            in0=G[:, (h - 1) * D:h * D],
                            in1=G[:, (cur - 1) * D:cur * D],
                            op=mybir.AluOpType.add,
                        )
                    cur = h
                o_t = rp.tile([P, D], mybir.dt.float32, tag="o")
                if weighted:
                    nc.vector.tensor_copy(out=o_t[:], in_=G[:, :D])
                else:
                    nc.vector.tensor_scalar_mul(out=o_t[:], in0=G[:, :D],
                                                scalar1=1.0 / K)
                nc.sync.dma_start(out=out[sl, :], in_=o_t[:])
    nc.compile()
    return nc


def _prep_general(edge_seg, edge_dst, B):
    E = edge_dst.shape[0]
    order = np.argsort(edge_seg, kind="stable")
    sseg = edge_seg[order].astype(np.int64)
    sdst = edge_dst[order].astype(np.int32)
    counts = np.bincount(sseg, minlength=B).astype(np.int64)
    K = max(int(counts.max()), 1) if E else 1
    starts = np.zeros(B, np.int64)
    np.cumsum(counts[:-1], out=starts[1:])
    pos = np.arange(E, dtype=np.int64) - np.repeat(starts, counts)
    idx_mat = np.zeros((B, K), np.int32)
    wts_mat = np.zeros((B, K), np.float32)
    idx_mat[sseg, pos] = sdst
    inv = np.zeros(B, np.float32)
    nz = counts > 0
    inv[nz] = 1.0 / counts[nz]
    wts_mat[sseg, pos] = inv[sseg]
    return idx_mat, wts_mat, K


def kernel(features, edge_seg, edge_dst, num_nodes=None, **_unused):
    features = np.ascontiguousarray(np.asarray(features, dtype=np.float32))
    edge_seg = np.asarray(edge_seg)
    edge_dst = np.asarray(edge_dst)
    E = int(edge_dst.shape[0])
    if num_nodes is not None:
        B = int(np.asarray(num_nodes))
    else:
        B = int(edge_seg.max()) + 1

    K = E // B if B and E % B == 0 else 0
    uniform = K > 0 and np.array_equal(
        edge_seg, np.repeat(np.arange(B, dtype=edge_seg.dtype), K))
    if uniform:
        idx_mat = np.ascontiguousarray(edge_dst.reshape(B, K).astype(np.int32))
        wts_mat = None
    else:
        idx_mat, wts_mat, K = _prep_general(edge_seg, edge_dst, B)

    npc = -(-B // N_CORES)           # nodes per core (ceil)
    n_tiles = -(-npc // P)
    nodes_pad = n_tiles * P
    weighted = wts_mat is not None

    in_maps = None
    if not weighted:
        try:
            features_bf = features.astype(_bf16())
            in_maps, plans = prep_v3_core_maps(features_bf, idx_mat, B,
                                               npc, n_tiles)
            nc = build_program_v3(plans)
            for m in in_maps:
                m.pop("_plan", None)
        except (ValueError, OverflowError):
            in_maps = None

    if in_maps is None:
        nc = build_program(n_tiles, K, weighted)
        in_maps = []
        for c in range(N_CORES):
            lo = c * npc
            hi = min(B, (c + 1) * npc)
            idx_c = np.zeros((nodes_pad, K), np.int32)
            if hi > lo:
                idx_c[:hi - lo] = idx_mat[lo:hi]
            m = {"features": features, "idx": idx_c}
            if weighted:
                w_c = np.zeros((nodes_pad, K), np.float32)
                if hi > lo:
                    w_c[:hi - lo] = wts_mat[lo:hi]
                m["wts"] = w_c
            in_maps.append(m)

    kw = dict(TRACE_KWARGS) if TRACE else {}
    res = run_bass_kernel_spmd(nc, in_maps, list(range(N_CORES)), **kw)
    global LAST_RESULT
    LAST_RESULT = res
    parts = []
    for c in range(N_CORES):
        lo = c * npc
        hi = min(B, (c + 1) * npc)
        if hi > lo:
            parts.append(res.results[c]["out"][:hi - lo])
    return np.concatenate(parts, axis=0)


def prep_v3_core_maps(features_bf, idx_mat, B, npc, n_tiles):
    """Per-core host prep.  The program layout is data-dependent and differs
    per core, but SPMD needs ONE program: merge the per-core plans into a
    shared super-layout (max blocks per (group, window, tile) across cores)
    and re-emit each core's meta in that layout."""
    K = idx_mat.shape[1]
    if not np.array_equal(idx_mat[:, 0], np.arange(B, dtype=idx_mat.dtype)):
        raise ValueError("v3 requires a self-loop first column")
    Kg = K - 1
    nodes_pad = n_tiles * P
    bf = _bf16()
    wbounds = [w * WINDOW for w in range(N_WINDOWS + 1)]
    wbounds[-1] = N_TOTAL + 1

    # pass 1: per (core, tile, window) entries + exact block counts
    core_ent = []
    blocks = np.zeros((N_CORES, n_tiles, N_WINDOWS), np.int64)
    sls = []
    for c in range(N_CORES):
        lo = c * npc
        hi = min(B, (c + 1) * npc)
        cols = np.full((nodes_pad, Kg), -1, np.int32)
        sl_c = np.zeros((nodes_pad, D), bf)
        if hi > lo:
            cols[:hi - lo] = idx_mat[lo:hi, 1:]
            n_sl = min(nodes_pad, N_TOTAL - lo)
            sl_c[:n_sl] = features_bf[lo:lo + n_sl]
        sls.append(sl_c)
        ent = {}
        for t in range(n_tiles):
            rf = cols[t * P:(t + 1) * P]
            r = rf.ravel().astype(np.int64)
            p = np.repeat(np.arange(P, dtype=np.int64), Kg)
            keep = r >= 0
            r, p = r[keep], p[keep]
            o = np.argsort(r, kind="stable")
            rs, ps = r[o], p[o]
            bounds = np.searchsorted(rs, wbounds)
            for w in range(N_WINDOWS):
                s0, s1 = bounds[w], bounds[w + 1]
                ent[(t, w)] = (rs[s0:s1] - w * WINDOW, ps[s0:s1])
                blocks[c, t, w] = -(-(s1 - s0) // P)
        core_ent.append(ent)
    # shared layout: max blocks across cores
    sblocks = blocks.max(axis=0)      # [n_tiles, N_WINDOWS]

    plan = _V3Plan()
    plan.n_tiles = n_tiles
    plan.scale = 1.0 / K
    metas = [[] for _ in range(N_CORES)]
    goff = 0
    tbg_max = 0
    meta_max = 0
    for g0 in range(0, n_tiles, GROUP_TILES):
        tiles = list(range(g0, min(g0 + GROUP_TILES, n_tiles)))
        gt = len(tiles)
        boff = {}
        cur = 0
        for w in range(N_WINDOWS):
            for t in tiles:
                boff[(t, w)] = cur
                cur += int(sblocks[t, w])
        tbg = cur
        tbg_max = max(tbg_max, tbg)
        # calls shared across cores
        call_runs = []
        for w in range(N_WINDOWS):
            run, run_nb = [], 0
            for t in tiles + [None]:
                nb_t = int(sblocks[t, w]) if t is not None else 0
                if run and (t is None or run_nb + nb_t > MAX_CALL_BLOCKS):
                    call_runs.append((w, run, run_nb))
                    run, run_nb = [], 0
                if t is not None and nb_t:
                    run.append(t)
                    run_nb += nb_t
        call_list = []
        iw = 0
        for (w, run, run_nb) in call_runs:
            call_list.append((w, boff[(run[0], w)], run_nb, iw))
            iw += run_nb * 8
        # nid columns (shared layout, per-core values)
        ncols = 2 * sum(int(sblocks[t, w])
                        for t in tiles for w in range(N_WINDOWS))
        tile_list = []
        col = 0
        colof = {}
        for ti, t in enumerate(tiles):
            mm = []
            for w in range(N_WINDOWS):
                nb_t = int(sblocks[t, w])
                for j in range(nb_t):
                    mm.append((boff[(t, w)] + j, iw + col))
                    colof[(t, w, j)] = col
                    col += 2
            tile_list.append((ti, mm))
        glen = iw + ncols
        # per-core meta piece
        for c in range(N_CORES):
            ent = core_ent[c]
            piece = np.zeros((128, glen), np.int16)
            for (w, run, run_nb), (w2, b0, nb, iw0) in \
                    zip(call_runs, call_list):
                e = np.zeros(run_nb * P, np.int16)
                off = 0
                for rt in run:
                    rel, _ = ent[(rt, w)]
                    e[off:off + rel.shape[0]] = rel.astype(np.int16)
                    off += int(sblocks[rt, w]) * P
                arr = e.reshape(run_nb * 8, 16).T
                piece[:, iw0:iw0 + run_nb * 8] = np.tile(arr, (8, 1))
            for t in tiles:
                for w in range(N_WINDOWS):
                    nb_t = int(sblocks[t, w])
                    if not nb_t:
                        continue
                    _, ps = ent[(t, w)]
                    pad = np.full(nb_t * P, -1.0, np.float32)
                    pad[:ps.shape[0]] = ps
                    colsarr = np.ascontiguousarray(
                        pad.reshape(nb_t, P).T).view(np.int16)
                    j0 = colof[(t, w, 0)]
                    piece[:, iw + j0:iw + j0 + 2 * nb_t] = colsarr
            metas[c].append(piece)
        plan.groups.append(dict(r0=g0 * P, gt=gt, goff=goff, glen=glen,
                                tbg=tbg, calls=call_list, tiles=tile_list))
        goff += glen
        meta_max = max(meta_max, glen)
    plan.tbg_max = tbg_max
    plan.meta_max = meta_max
    in_maps = []
    for c in range(N_CORES):
        meta_c = np.ascontiguousarray(np.concatenate(metas[c], axis=1))
        in_maps.append({"features": features_bf, "selfloop": sls[c],
                        "meta": meta_c})
    plan.meta = in_maps[0]["meta"]
    return in_maps, plan
